# revision 4
# baseline (speedup 1.0000x reference)
"""AlexNet-style CNN forward pass on 8 Trainium2 NeuronCores (bf16).

Key HW facts driving the design (measured on-device):
  - bf16 matmul streams ~1 elem/cycle only when the moving operand is
    CONTIGUOUS (N=512 -> 223ns/MM); strided windows (8-elem runs) run 4x
    slower, fp32r runs 3.3x slower at any shape.
  - So every conv matmul reads ONE contiguous run: for kernel offset
    (dy,dx) the rhs starts at linear offset dy*W+dx and the contribution
    of offset (dy,dx) to output pixel (Y,X) lands at psum column
    (Y*W+X) - (dy*W+dx) + const, independent of the offset.  Pad columns
    produce garbage psum columns that the evict simply skips.
  - Images are packed with SHARED zero pad rows so one matmul covers
    several images per PSUM bank (512 fp32 columns).

Layout strategy:
  - Convs data-parallel (32 img/core), channels on partitions, weights
    replicated.  conv1 via host-packed im2col (K=100 incl. bias row),
    output channels duplicated to M=128 so conv2's x+1-shifted duplicate
    is written by partition-aligned evicts (no cross-partition DMA).
  - conv2: K=128 = 64ch x {x, x+1}; 28 uniform matmuls per (image, mc).
  - conv3: ch0:128 aligned (X3a) + [ch128:192 ; ch128:192 @ x+1] (X3b)
    so the 64-channel tail also runs as full K=128 matmuls.
  - FC model-parallel (512 rows of fc1/fc2, 512 K of fc3 per core) with
    chunked bf16 AllGathers overlapped with compute; fc3 AllReduced.
"""

import numpy as np
import ml_dtypes

import concourse.bass as bass
import concourse.mybir as mybir
import concourse.tile as tile
from concourse import bacc
from concourse.bass_utils import run_bass_kernel_spmd

N_CORES = 8
B = 256
BC = B // N_CORES  # 32 images per core

F32 = mybir.dt.float32
BF16 = mybir.dt.bfloat16
RELU = mybir.ActivationFunctionType.Relu
IDENT = mybir.ActivationFunctionType.Identity
BF = ml_dtypes.bfloat16

# linear-packed activation buffer geometry
W2, S2 = 22, 484            # conv2 input: 22x22 per image, no sharing
W3, S3 = 12, 120            # conv3 input: 12 wide, images share 2 pad rows
W4, S4 = 10, 90             # conv4/5 input: 10 wide, share 1 pad row
N3 = BC * S3 + 24           # X3a/X3b free size (3864)
N4 = BC * S4 + 20           # X4/X5 free size (2900)
SP2 = 15 * W2 + 16          # conv2 psum span per image (346)
SP3 = 3 * S3 + 7 * W3 + 8   # conv3 psum span, 4 images (452)
SP4 = 3 * S4 + 7 * W4 + 8   # conv4/5 psum span, 4 images (348)


def _emit(nc, tc, t, yout):
    sync = nc.sync
    act = nc.scalar
    dve = nc.vector
    pool_e = nc.gpsimd

    psum = tc.alloc_tile_pool(name="psum", bufs=6, space="PSUM")
    scr = tc.alloc_tile_pool(name="scr", bufs=1, side="left")
    dram = tc.alloc_tile_pool(name="dram", bufs=1, space="DRAM")

    # ---------------- left-stack pools: conv1/conv2 era ------------------
    p_w12 = tc.alloc_tile_pool(name="p_w12", bufs=1, side="left")
    p_x2 = tc.alloc_tile_pool(name="p_x2", bufs=1, side="left")
    p_x13 = tc.alloc_tile_pool(name="p_x13", bufs=2, side="left")

    lw1 = p_w12.tile([100, 4 * 128], BF16)
    sync.dma_start(lw1[:], t["lw1"][:])
    lw2 = p_w12.tile([128, 7 * 4 * 256], BF16)
    sync.dma_start(lw2[:], t["lw2"][:])
    lb2 = p_w12.tile([128, 2], F32)
    sync.dma_start(lb2[:], t["lb2"][:])

    # conv2 input, linear: [128, 32 img * 484]; per image 22y x 22x, image
    # pixel (y,x) at Y=y+3, X=x+3.  rows 0:64 ch c, rows 64:128 ch c @ x+1
    X2s = p_x2.tile([128, BC * S2], BF16)
    pool_e.memset(X2s[:].bitcast(F32), 0.0)

    # ---------------- conv1 + pool1 ----------------
    _sid = nc.enter_named_scope("L1_conv1", False)[0]
    for bg in range(8):  # groups of 4 images
        xt = p_x13.tile([100, 4 * 41 * 32], BF16, tag="x13")
        sync.dma_start(xt[:], t["x13"][bg])
        xtv = xt.rearrange("k (b y x) -> k b y x", b=4, y=41, x=32)
        for bl in range(4):
            b = bg * 4 + bl
            for h in range(2):  # vertical half of the 32x32 output
                ps = psum.tile([128, 512], F32, tag="ps")
                for pi in range(4):
                    nc.tensor.matmul(
                        ps[:],
                        lw1[:, pi * 128:(pi + 1) * 128],
                        xtv[:, bl, h * 16 + 3 * pi:h * 16 + 3 * pi + 16, :],
                        start=(pi == 0), stop=(pi == 3),
                    )
                psv = ps.rearrange("m (y x) -> m y x", y=16, x=32)
                m1 = scr.tile([128, 256], BF16, tag="m1", bufs=3)
                m2 = scr.tile([128, 128], BF16, tag="m2", bufs=3)
                dve.tensor_max(m1.rearrange("m (y x) -> m y x", y=16, x=16),
                               psv[:, :, 0::2], psv[:, :, 1::2])
                m1v = m1.rearrange("m (y x) -> m y x", y=16, x=16)
                pool_e.tensor_max(m2.rearrange("m (y x) -> m y x", y=8, x=16),
                                  m1v[:, 0::2, :], m1v[:, 1::2, :])
                m2v = m2.rearrange("m (y x) -> m y x", y=8, x=16)
                # relu (bias came in via the ones-row); lo half at X=3..18,
                # hi half (the x+1-shifted copy) one column to the left
                off = b * S2 + (h * 8 + 3) * W2
                act.activation(
                    X2s[0:64, off + 3:off + 3 + 8 * W2]
                    .rearrange("p (y x) -> p y x", x=W2)[:, :, 0:16],
                    m2v[0:64], RELU)
                pool_e.tensor_scalar_max(
                    X2s[64:128, off + 2:off + 2 + 8 * W2]
                    .rearrange("p (y x) -> p y x", x=W2)[:, :, 0:16],
                    m2v[64:128], 0.0)
    p_x13.release()
    nc.leave_named_scope("L1_conv1", _sid, False)

    # conv3 weights (prefetch during conv2) + conv3 input buffers
    p_w3 = tc.alloc_tile_pool(name="p_w3", bufs=1, side="right")
    p_x3 = tc.alloc_tile_pool(name="p_x3", bufs=1, side="right")
    lw3a = p_w3.tile([128, 25 * 384], BF16)
    sync.dma_start(lw3a[:], t["lw3a"][:])
    lw3b = p_w3.tile([128, 15 * 384], BF16)
    sync.dma_start(lw3b[:], t["lw3b"][:])
    lb3 = p_w3.tile([128, 3], F32)
    sync.dma_start(lb3[:], t["lb3"][:])
    X3a = p_x3.tile([128, N3], BF16)
    X3b = p_x3.tile([128, N3], BF16)
    pool_e.memset(X3a[:].bitcast(F32), 0.0)
    pool_e.memset(X3b[:].bitcast(F32), 0.0)

    # ---------------- conv2 + pool2 (linear, per-image psum) -------------
    _sid = nc.enter_named_scope("L2_conv2", False)[0]
    lw2v = lw2.rearrange("k (d j m) -> k d j m", d=7, j=4, m=256)
    for b in range(BC):
        for mc in range(2):
            ps = psum.tile([128, 512], F32, tag="ps")
            for dy in range(7):
                for j in range(4):
                    nc.tensor.matmul(
                        ps[:, 0:SP2],
                        lw2v[:, dy, j, mc * 128:mc * 128 + 128],
                        X2s[:, b * S2 + dy * W2 + 2 * j:
                            b * S2 + dy * W2 + 2 * j + SP2],
                        start=(dy == 0 and j == 0), stop=(dy == 6 and j == 3),
                    )
            # psum col (y*22 + x) = out pixel (y,x); pool 2x2 + relu + bias
            m1 = scr.tile([128, 128], BF16, tag="m1", bufs=3)
            m2 = scr.tile([128, 64], BF16, tag="m2", bufs=3)
            p2a = ps[:, 0:SP2].rearrange("m f -> m f")
            dve.tensor_max(
                m1.rearrange("m (y x) -> m y x", y=16, x=8),
                ps[:, 0:16 * W2].rearrange("m (y x) -> m y x", y=16, x=W2)[
                    :, :, 0:16:2],
                ps[:, 0:16 * W2].rearrange("m (y x) -> m y x", y=16, x=W2)[
                    :, :, 1:16:2])
            m1v = m1.rearrange("m (y x) -> m y x", y=16, x=8)
            pool_e.tensor_max(m2.rearrange("m (y x) -> m y x", y=8, x=8),
                              m1v[:, 0::2, :], m1v[:, 1::2, :])
            m2v = m2.rearrange("m (y x) -> m y x", y=8, x=8)
            if mc == 0:
                act.activation(
                    X3a[:, b * S3 + 2 * W3 + 2:b * S3 + 10 * W3]
                    .rearrange("p (y x) -> p y x", y=8)[:, :, 0:8],
                    m2v[:], RELU, bias=lb2[:, 0:1])
            else:
                act.activation(
                    X3b[0:64, b * S3 + 2 * W3 + 2:b * S3 + 10 * W3]
                    .rearrange("p (y x) -> p y x", y=8)[:, :, 0:8],
                    m2v[0:64], RELU, bias=lb2[0:64, 1:2])
                act.activation(
                    X3b[64:128, b * S3 + 2 * W3 + 1:b * S3 + 10 * W3 - 1]
                    .rearrange("p (y x) -> p y x", y=8)[:, :, 0:8],
                    m2v[64:128], RELU, bias=lb2[64:128, 1:2])
    nc.leave_named_scope("L2_conv2", _sid, False)
    p_x2.release()
    p_w12.release()

    # conv4/5 weights (prefetch during conv3) + conv4 input buffers
    p_w45 = tc.alloc_tile_pool(name="p_w45", bufs=1, side="left")
    p_x4 = tc.alloc_tile_pool(name="p_x4", bufs=1, side="left")
    lw4 = p_w45.tile([128, 27 * 256], BF16)
    sync.dma_start(lw4[:], t["lw4"][:])
    lb4 = p_w45.tile([128, 2], F32)
    sync.dma_start(lb4[:], t["lb4"][:])
    lw5 = p_w45.tile([128, 18 * 256], BF16)
    sync.dma_start(lw5[:], t["lw5"][:])
    lb5 = p_w45.tile([128, 2], F32)
    sync.dma_start(lb5[:], t["lb5"][:])
    X4 = []
    for i in range(3):
        X4.append(p_x4.tile([128, N4], BF16, name=f"X4_{i}"))
        pool_e.memset(X4[i][:].bitcast(F32), 0.0)

    _sid = nc.enter_named_scope("L3_conv3", False)[0]
    # ---------------- conv3 (4-image chunks) ----------------
    lw3av = lw3a.rearrange("k (o m) -> k o m", o=25)
    lw3bv = lw3b.rearrange("k (s m) -> k s m", s=15)
    for ch in range(8):
        b0 = ch * 4
        for mc in range(3):
            ps = psum.tile([128, 512], F32, tag="ps")
            for o in range(25):
                dy, dx = o // 5, o % 5
                nc.tensor.matmul(
                    ps[:, 0:SP3], lw3av[:, o, mc * 128:mc * 128 + 128],
                    X3a[:, b0 * S3 + dy * W3 + dx:
                        b0 * S3 + dy * W3 + dx + SP3],
                    start=(o == 0), stop=False)
            for s in range(15):
                dy, dx = s // 3, 2 * (s % 3)
                nc.tensor.matmul(
                    ps[:, 0:SP3], lw3bv[:, s, mc * 128:mc * 128 + 128],
                    X3b[:, b0 * S3 + dy * W3 + dx:
                        b0 * S3 + dy * W3 + dx + SP3],
                    start=False, stop=(s == 14))
            # psum col (b*120 + y*12 + x) = img b out pixel (y,x)
            act.activation(
                X4[mc][:, b0 * S4 + S4 + 11:b0 * S4 + 4 * S4]
                .rearrange("p (b f) -> p b f", b=4)[:, :, 0:7 * W4 + 8]
                .rearrange("p b (y x) -> p b y x", x=W4)[:, :, 0:8, 0:8],
                ps[:, 2 * W3 + 2:SP3 + 2 * W3 + 2 - 26]
                .rearrange("p (b f) -> p b f", b=4)[:, :, 0:7 * W3 + 8]
                .rearrange("p b (y x) -> p b y x", x=W3)[:, :, 0:8, 0:8],
                RELU, bias=lb3[:, mc:mc + 1])
    nc.leave_named_scope("L3_conv3", _sid, False)
    p_x3.release()
    p_w3.release()

    # fc1 weights (prefetch during conv4) + conv5 input buffers
    p_fw1 = tc.alloc_tile_pool(name="p_fw1", bufs=1, side="right")
    p_x5 = tc.alloc_tile_pool(name="p_x5", bufs=1, side="right")
    fw1 = p_fw1.tile([128, 32 * 512], BF16)
    sync.dma_start(fw1[:], t["fw1s"][:])
    fb1 = p_fw1.tile([128, 4], F32)
    sync.dma_start(fb1[:], t["fb1s"][:])
    X5 = []
    for i in range(2):
        X5.append(p_x5.tile([128, N4], BF16, name=f"X5_{i}"))
        pool_e.memset(X5[i][:].bitcast(F32), 0.0)

    def ps4_view(ps):
        # psum [128, SP4] -> [p, 4 img, 8, 8] interior view
        return (ps[:, W4 + 1:SP4 + W4 + 1 - 12]
                .rearrange("p (b f) -> p b f", b=4)[:, :, 0:7 * W4 + 8]
                .rearrange("p b (y x) -> p b y x", x=W4)[:, :, 0:8, 0:8])

    def x45_win(xb_, b0):
        return (xb_[:, b0 * S4 + S4 + 11:b0 * S4 + 4 * S4]
                .rearrange("p (b f) -> p b f", b=4)[:, :, 0:7 * W4 + 8]
                .rearrange("p b (y x) -> p b y x", x=W4)[:, :, 0:8, 0:8])

    _sid = nc.enter_named_scope("L4_conv4", False)[0]
    # ---------------- conv4 (4-image chunks) ----------------
    lw4v = lw4.rearrange("k (o m) -> k o m", o=27)
    for ch in range(8):
        b0 = ch * 4
        for mc in range(2):
            ps = psum.tile([128, 512], F32, tag="ps")
            first = True
            for dy in range(3):
                for dx in range(3):
                    for kc in range(3):
                        o = (dy * 3 + dx) * 3 + kc
                        nc.tensor.matmul(
                            ps[:, 0:SP4],
                            lw4v[:, o, mc * 128:mc * 128 + 128],
                            X4[kc][:, b0 * S4 + dy * W4 + dx:
                                   b0 * S4 + dy * W4 + dx + SP4],
                            start=first, stop=(o == 26),
                        )
                        first = False
            act.activation(x45_win(X5[mc], b0), ps4_view(ps),
                           RELU, bias=lb4[:, mc:mc + 1])
    nc.leave_named_scope("L4_conv4", _sid, False)
    p_x4.release()

    # fc2/fc3 weights (prefetch during conv5)
    p_fw2 = tc.alloc_tile_pool(name="p_fw2", bufs=1, side="left")
    fw2 = p_fw2.tile([128, 32 * 512], BF16)
    sync.dma_start(fw2[:], t["fw2s"][:])
    fb2 = p_fw2.tile([128, 4], F32)
    sync.dma_start(fb2[:], t["fb2s"][:])
    fw3 = p_fw2.tile([128, 4 * 100], BF16)
    sync.dma_start(fw3[:], t["fw3s"][:])
    fb3 = p_fw2.tile([100, 1], F32)
    sync.dma_start(fb3[:], t["fb3s"][:])

    # pool5 -> DRAM staging (2 chunks of 16 images for the AllGather)
    cin5 = dram.tile([2, 2, 128, 256], BF16)  # [chunk, mc, c, 16img*16yx]
    g1 = dram.tile([2, N_CORES, 2, 128, 256], BF16)

    _sid = nc.enter_named_scope("L5_conv5", False)[0]
    # ---------------- conv5 + pool5 + chunked AllGather ----------------
    lw5v = lw5.rearrange("k (o m) -> k o m", o=18)
    for ch in range(8):
        b0 = ch * 4
        for mc in range(2):
            ps = psum.tile([128, 512], F32, tag="ps")
            first = True
            for dy in range(3):
                for dx in range(3):
                    for kc in range(2):
                        o = (dy * 3 + dx) * 2 + kc
                        nc.tensor.matmul(
                            ps[:, 0:SP4],
                            lw5v[:, o, mc * 128:mc * 128 + 128],
                            X5[kc][:, b0 * S4 + dy * W4 + dx:
                                   b0 * S4 + dy * W4 + dx + SP4],
                            start=first, stop=(o == 17),
                        )
                        first = False
            psv = ps4_view(ps)
            m1 = scr.tile([128, 128], BF16, tag="m1", bufs=3)
            m2 = scr.tile([128, 64], BF16, tag="m2", bufs=3)
            dve.tensor_max(m1.rearrange("m (b y x) -> m b y x", b=4, y=8, x=4),
                           psv[:, :, :, 0::2], psv[:, :, :, 1::2])
            m1v = m1.rearrange("m (b y x) -> m b y x", b=4, y=8, x=4)
            pool_e.tensor_max(m2.rearrange("m (b y x) -> m b y x", b=4, y=4, x=4),
                              m1v[:, :, 0::2, :], m1v[:, :, 1::2, :])
            p5t = scr.tile([128, 64], BF16, tag="p5t", bufs=2)
            act.activation(p5t[:], m2[:], RELU, bias=lb5[:, mc:mc + 1])
            sync.dma_start(
                cin5[ch // 4, mc, :, (ch % 4) * 64:(ch % 4) * 64 + 64],
                p5t[:])
        if ch % 4 == 3:
            h = ch // 4
            pool_e.collective_compute(
                "AllGather", mybir.AluOpType.bypass,
                replica_groups=[list(range(N_CORES))],
                ins=[cin5[h].opt()], outs=[g1[h].opt()])
    nc.leave_named_scope("L5_conv5", _sid, False)
    p_x5.release()
    p_w45.release()

    _sid = nc.enter_named_scope("G1_gather", False)[0]
    # ---- assemble fc1 input: DMA to (r, img, yx), DVE-transpose to
    # (yx, img) so fc1's moving operands are contiguous 256-wide ---------
    p_h1 = tc.alloc_tile_pool(name="p_h1", bufs=1, side="right")
    H1r = [p_h1.tile([128, N_CORES * BC * 16], BF16, name=f"H1r_{i}")
           for i in range(2)]
    H1 = [p_h1.tile([128, 16 * B], BF16, name=f"H1_{i}") for i in range(2)]
    for h in range(2):
        for cc in range(2):
            sync.dma_start(
                H1r[cc].rearrange("c (r s) -> c r s", r=N_CORES)[
                    :, :, h * 256:(h + 1) * 256],
                g1[h, :, cc].rearrange("r c f -> c r f"))
    for cc in range(2):
        for r in range(N_CORES):
            dve.tensor_copy(
                H1[cc].rearrange("c (y i) -> c y i", y=16)[
                    :, :, r * BC:(r + 1) * BC],
                H1r[cc].rearrange("c (r i y) -> c r i y", r=N_CORES, y=16)[
                    :, r].rearrange("c i y -> c y i"))
    nc.leave_named_scope("G1_gather", _sid, False)

    _sid = nc.enter_named_scope("F1_fc1", False)[0]
    # ---------------- fc1 (model-parallel over 512 outputs) --------------
    p_f1 = tc.alloc_tile_pool(name="p_f1", bufs=1, side="left")
    F1 = p_f1.tile([128, 4 * B], BF16)
    cin6 = dram.tile([2, 128, 512], BF16)
    g2 = dram.tile([2, N_CORES, 128, 512], BF16)
    fw1v = fw1.rearrange("k (y c m) -> k y c m", y=16, c=2, m=512)
    for mc in range(4):
        ps = psum.tile([128, 512], F32, tag="ps")
        first = True
        for yx in range(16):
            for cc in range(2):
                nc.tensor.matmul(
                    ps[:, 0:B],
                    fw1v[:, yx, cc, mc * 128:mc * 128 + 128],
                    H1[cc][:, yx * B:(yx + 1) * B],
                    start=first, stop=(yx == 15 and cc == 1))
                first = False
        act.activation(F1[:, mc * B:(mc + 1) * B], ps[:, 0:B], RELU,
                       bias=fb1[:, mc:mc + 1])
        if mc % 2 == 1:
            p = mc // 2
            sync.dma_start(cin6[p], F1[:, p * 512:(p + 1) * 512])
            pool_e.collective_compute(
                "AllGather", mybir.AluOpType.bypass,
                replica_groups=[list(range(N_CORES))],
                ins=[cin6[p].opt()], outs=[g2[p].opt()])
    p_h1.release()
    p_fw1.release()
    nc.leave_named_scope("F1_fc1", _sid, False)

    _sid = nc.enter_named_scope("G2_gather", False)[0]
    # ---------------- assemble fc2 input [c, (r, mc, img)] ---------------
    p_h2 = tc.alloc_tile_pool(name="p_h2", bufs=1, side="right")
    H2 = p_h2.tile([128, N_CORES * 4 * B], BF16)
    for p in range(2):
        sync.dma_start(
            H2.rearrange("c (r q f) -> c r q f", r=N_CORES, q=2)[:, :, p, :],
            g2[p].rearrange("r c f -> c r f"))
    nc.leave_named_scope("G2_gather", _sid, False)

    _sid = nc.enter_named_scope("F2_fc2", False)[0]
    # ---------------- fc2 ----------------
    p_f2 = tc.alloc_tile_pool(name="p_f2", bufs=1, side="left")
    F2 = p_f2.tile([128, 4 * B], BF16)
    fw2v = fw2.rearrange("k (a m) -> k a m", a=32)
    for mc in range(4):
        ps = psum.tile([128, 512], F32, tag="ps")
        for kc in range(32):
            nc.tensor.matmul(
                ps[:, 0:B], fw2v[:, kc, mc * 128:mc * 128 + 128],
                H2[:, kc * B:(kc + 1) * B],
                start=(kc == 0), stop=(kc == 31))
        act.activation(F2[:, mc * B:(mc + 1) * B], ps[:, 0:B], RELU,
                       bias=fb2[:, mc:mc + 1])
    p_h2.release()
    nc.leave_named_scope("F2_fc2", _sid, False)

    _sid = nc.enter_named_scope("F3_fc3", False)[0]
    # ---------------- fc3 (partial over this core's 512 K) + AllReduce ---
    fw3v = fw3.rearrange("k (a m) -> k a m", a=4)
    ps = psum.tile([128, 512], F32, tag="ps")
    for kc in range(4):
        nc.tensor.matmul(
            ps[0:100, 0:B], fw3v[:, kc, :], F2[:, kc * B:(kc + 1) * B],
            start=(kc == 0), stop=(kc == 3))
    s3 = scr.tile([128, 512], F32, tag="ev", bufs=2)
    s3v = s3[0:100, 0:B]
    act.activation(s3v, ps[0:100, 0:B], IDENT, bias=fb3[:])  # + fb3/8
    cin7 = dram.tile([100, B], F32)
    sync.dma_start(cin7[:], s3v)
    g3 = dram.tile([100, B], F32)
    pool_e.collective_compute(
        "AllReduce", mybir.AluOpType.add,
        replica_groups=[list(range(N_CORES))],
        ins=[cin7.opt()], outs=[g3.opt()])
    sync.dma_start(yout[:], g3[:])
    nc.leave_named_scope("F3_fc3", _sid, False)
    p_f2.release()
    p_f1.release()
    p_fw2.release()

    scr.release()
    dram.release()
    psum.release()


# ---------------------------------------------------------------------------
# host-side input prep (numpy; all weight arrays already in SBUF layout)
# ---------------------------------------------------------------------------

def _prep_shared(w1, b1, w2, b2, w3, b3, w4, b4, w5, b5):
    f = np.float32
    # conv1: rows r = dyo*33 + dx*3 + c, row 99 = bias(ones); M=128 dup'd
    lw1 = np.zeros((100, 4 * 128), f)
    for p in range(4):
        for dyo in range(3):
            dy = 3 * p + dyo
            if dy > 10:
                continue
            for dx in range(11):
                for c in range(3):
                    r = dyo * 33 + dx * 3 + c
                    lw1[r, p * 128:p * 128 + 64] = w1[:, c, dy, dx]
                    lw1[r, p * 128 + 64:p * 128 + 128] = w1[:, c, dy, dx]
    lw1[99, 0:64] = b1
    lw1[99, 64:128] = b1

    def mexp(wt):  # [192 out, 64 in] -> [64, 256] with ch128:192 duplicated
        return np.concatenate([wt[0:128].T, wt[128:192].T, wt[128:192].T],
                              axis=1)

    # conv2: [128, (dy7, j4, m256)]: rows 0:64 = w @ dx=2j, 64:128 @ dx=2j+1
    lw2 = np.zeros((128, 7 * 4 * 256), f)
    for dy in range(7):
        for j in range(4):
            blk = dy * 4 + j
            lw2[0:64, blk * 256:(blk + 1) * 256] = mexp(w2[:, :, dy, 2 * j])
            if 2 * j + 1 < 7:
                lw2[64:128, blk * 256:(blk + 1) * 256] = \
                    mexp(w2[:, :, dy, 2 * j + 1])
    lb2 = np.zeros((128, 2), f)
    lb2[:, 0] = b2[0:128]
    lb2[:, 1] = np.concatenate([b2[128:192], b2[128:192]])

    # conv3: A [128, (o25, m384)] = ch0:128; B [128, (s15, m384)]:
    # s = dy*3 + jj, dx = 2*jj: rows 0:64 H2 @ dx, rows 64:128 H2 @ dx+1
    lw3a = np.zeros((128, 25 * 384), f)
    for o in range(25):
        dy, dx = o // 5, o % 5
        lw3a[:, o * 384:(o + 1) * 384] = w3[:, 0:128, dy, dx].T
    lw3b = np.zeros((128, 15 * 384), f)
    for s in range(15):
        dy, dx = s // 3, 2 * (s % 3)
        lw3b[0:64, s * 384:(s + 1) * 384] = w3[:, 128:192, dy, dx].T
        if dx + 1 < 5:
            lw3b[64:128, s * 384:(s + 1) * 384] = w3[:, 128:192, dy, dx + 1].T
    lb3 = np.zeros((128, 3), f)
    lb3[:, 0] = b3[0:128]; lb3[:, 1] = b3[128:256]; lb3[:, 2] = b3[256:384]

    # conv4 / conv5: [128, (o, m)] with o = (dy*3+dx)*nkc + kc
    lw4 = np.zeros((128, 27 * 256), f)
    for dy in range(3):
        for dx in range(3):
            for kc in range(3):
                o = (dy * 3 + dx) * 3 + kc
                lw4[:, o * 256:(o + 1) * 256] = w4[:, kc * 128:(kc + 1) * 128, dy, dx].T
    lb4 = np.stack([b4[0:128], b4[128:256]], axis=1).astype(f)
    lw5 = np.zeros((128, 18 * 256), f)
    for dy in range(3):
        for dx in range(3):
            for kc in range(2):
                o = (dy * 3 + dx) * 2 + kc
                lw5[:, o * 256:(o + 1) * 256] = w5[:, kc * 128:(kc + 1) * 128, dy, dx].T
    lb5 = np.stack([b5[0:128], b5[128:256]], axis=1).astype(f)
    return dict(lw1=lw1.astype(BF), lw2=lw2.astype(BF), lb2=lb2,
                lw3a=lw3a.astype(BF), lw3b=lw3b.astype(BF), lb3=lb3,
                lw4=lw4.astype(BF), lb4=lb4, lw5=lw5.astype(BF), lb5=lb5)


def _prep_x13(x):
    """x [B,3,32,32] -> per-core [8, 100, 4*41*32] im2col-packed bf16."""
    f = np.float32
    xpad = np.zeros((B, 3, 44, 42), f)
    xpad[:, :, 5:37, 5:37] = x
    X = np.zeros((100, B, 41, 32), f)
    for dyo in range(3):
        for dx in range(11):
            for c in range(3):
                X[dyo * 33 + dx * 3 + c] = xpad[:, c, dyo:dyo + 41, dx:dx + 32]
    X[99] = 1.0
    out = []
    for r in range(N_CORES):
        pc = X[:, r * BC:(r + 1) * BC]  # [100, 32, 41, 32]
        pc = pc.reshape(100, 8, 4 * 41 * 32).transpose(1, 0, 2)
        out.append(np.ascontiguousarray(pc.astype(BF)))
    return out


def _prep_fc(fw1, fb1, fw2, fb2, fw3, fb3):
    f = np.float32
    outs = []
    for r in range(N_CORES):
        sl = slice(512 * r, 512 * (r + 1))
        # fw1s [128, (yx, cc, m)]: fw1[512r+m, (cc*128+k)*16+yx]
        fw1s = fw1[sl].reshape(512, 2, 128, 16).transpose(2, 3, 1, 0).reshape(128, -1)
        fb1s = fb1[sl].reshape(4, 128).T
        # fw2s [128, (kc, m)]: fw2[512r+m, kc*128+k]
        fw2s = fw2[sl].reshape(512, 32, 128).transpose(2, 1, 0).reshape(128, -1)
        fb2s = fb2[sl].reshape(4, 128).T
        # fw3s [128, (kc, m)]: fw3[m, 512r + kc*128 + k]
        fw3s = fw3[:, sl].reshape(100, 4, 128).transpose(2, 1, 0).reshape(128, -1)
        fb3s = (fb3 / N_CORES).reshape(100, 1)
        outs.append(dict(
            fw1s=np.ascontiguousarray(fw1s.astype(BF)),
            fb1s=np.ascontiguousarray(fb1s.astype(f)),
            fw2s=np.ascontiguousarray(fw2s.astype(BF)),
            fb2s=np.ascontiguousarray(fb2s.astype(f)),
            fw3s=np.ascontiguousarray(fw3s.astype(BF)),
            fb3s=np.ascontiguousarray(fb3s.astype(f)),
        ))
    return outs


_CACHE = {}

_SHAPES = dict(
    x13=(8, 100, 4 * 41 * 32), lw1=(100, 4 * 128),
    lw2=(128, 7 * 4 * 256), lb2=(128, 2),
    lw3a=(128, 25 * 384), lw3b=(128, 15 * 384), lb3=(128, 3),
    lw4=(128, 27 * 256), lb4=(128, 2),
    lw5=(128, 18 * 256), lb5=(128, 2),
    fw1s=(128, 32 * 512), fb1s=(128, 4),
    fw2s=(128, 32 * 512), fb2s=(128, 4),
    fw3s=(128, 4 * 100), fb3s=(100, 1),
)

_BF16_INPUTS = {"x13", "lw1", "lw2", "lw3a", "lw3b", "lw4", "lw5",
                "fw1s", "fw2s", "fw3s"}


def _build():
    if "nc" in _CACHE:
        return _CACHE["nc"]
    nc = bacc.Bacc("TRN2", target_bir_lowering=False, debug=False,
                   num_devices=N_CORES)
    t = {name: nc.dram_tensor(
            name, list(shape), BF16 if name in _BF16_INPUTS else F32,
            kind="ExternalInput").ap()
         for name, shape in _SHAPES.items()}
    yout = nc.dram_tensor("yout", [100, B], F32, kind="ExternalOutput").ap()
    with tile.TileContext(nc) as tc:
        _emit(nc, tc, t, yout)
    nc.compile()
    _CACHE["nc"] = nc
    return nc


def kernel(x, w1, b1, w2, b2, w3, b3, w4, b4, w5, b5,
           fw1, fb1, fw2, fb2, fw3, fb3):
    args = [np.asarray(a, np.float32) for a in
            (x, w1, b1, w2, b2, w3, b3, w4, b4, w5, b5, fw1, fb1, fw2, fb2, fw3, fb3)]
    (x, w1, b1, w2, b2, w3, b3, w4, b4, w5, b5,
     fw1, fb1, fw2, fb2, fw3, fb3) = args
    nc = _build()
    shared = _prep_shared(w1, b1, w2, b2, w3, b3, w4, b4, w5, b5)
    x13s = _prep_x13(x)
    fcs = _prep_fc(fw1, fb1, fw2, fb2, fw3, fb3)
    in_maps = [{**shared, "x13": x13s[r], **fcs[r]} for r in range(N_CORES)]
    res = run_bass_kernel_spmd(nc, in_maps, list(range(N_CORES)))
    y = res.results[0]["yout"]  # [100, 256]
    return np.ascontiguousarray(np.asarray(y, np.float32).T)
